# revision 4
# baseline (speedup 1.0000x reference)
"""Trainium2 Bass kernel for nn_Interpolator (quadratic-form kernel interpolation).

Math (T=8192 targets, C=8192 contexts, D=64, DY=32):
    S = W + W^T
    scores[t,c] = (z_t - z_c)^T W (z_t - z_c)
                = q_tt[t] + q_cc[c] - z_t^T S z_c
    theta = exp(-scores);  out = (theta @ y_context) / theta.sum(-1, keepdim)

q_tt[t] scales whole theta rows and cancels in the normalization -> dropped.
q_cc[c] = 0.5 * z_c^T S z_c is folded into the main matmul contraction:
the stationary operand LC has 128 rows: rows 0..63 = zc^T, rows 64..127 =
0.5*(zc .* (S zc)); the moving operand RT has rows 0..63 = S^T z_t and
rows 64..127 = -1. A single K=128 fp16 matmul then yields
cross - q_cc directly (matmul cost depends only on moving columns, so the
q_cc fold is free on the PE).

Sharding: data-parallel over targets; each of the 8 cores takes T/8 = 1024
targets and the full context set.

Per-core device program:
  - main loop over 64 context chunks of 128:
      sc[128,1024]  = LC[:,chunk]^T @ RT          (one 1024-wide fp16 matmul)
      th[128,1024]  = Exp(sc)                      (ACT, bf16 out; the roofline)
      o2[33,1024]  += YA[:,j,:]^T @ th             (one 1024-wide bf16 matmul)
    y_aug col 32 = ones gives the denominator row.
  - the 16 zs pieces (rows 64..127 of LC) are computed in the prelude /
    interleaved every 4th chunk, riding PE+DVE slack.
  - input DMAs are issued up front, split across the SP and ACT HWDGE
    queues; zc is split so chunk 0's block lands first and the loop can
    start while the rest streams in.
  - a short dependency-free matmul burst flips the PE HAM clock-gate to
    8/8 early; a dummy Exp preloads the ACT spline table.
Host: shard/transpose/cast inputs (layout only), concat per-core [33,1024]
outputs, divide numerator rows by the denominator row.
"""

import ml_dtypes
import numpy as np

import concourse.bacc as bacc
import concourse.bass as bass
import concourse.mybir as mybir
import concourse.tile as tile
from concourse.bass_utils import run_bass_kernel_spmd

F32 = mybir.dt.float32
F16 = mybir.dt.float16
BF16 = mybir.dt.bfloat16

T, C, D, DY = 8192, 8192, 64, 32
NCORES = 8
TL = T // NCORES          # 1024 targets per core
NCHUNK = C // 128         # 64 context chunks of 128
NPIECE = C // 512         # 16 zs pieces of 512 contexts
NWARM = 6


def _build_kernel_body(tc: tile.TileContext):
    nc = tc.nc
    Exp = mybir.ActivationFunctionType.Exp

    wwt_d = nc.dram_tensor("wwt", [D, 2 * D], F32, kind="ExternalInput")
    zt_d = nc.dram_tensor("ztt", [D, TL], F16, kind="ExternalInput")
    zc0_d = nc.dram_tensor("zc0", [D, 1024], F16, kind="ExternalInput")
    zc13_d = nc.dram_tensor("zc13", [D, 3072], F16, kind="ExternalInput")
    zc47_d = nc.dram_tensor("zc47", [D, 4096], F16, kind="ExternalInput")
    y_d = nc.dram_tensor("yck", [128, NCHUNK * DY], BF16, kind="ExternalInput")
    out_d = nc.dram_tensor("out", [DY + 1, TL], F32, kind="ExternalOutput")

    with (
        tc.tile_pool(name="sb", bufs=1) as sb,
        tc.tile_pool(name="pp", bufs=1, space="PSUM") as pp,
    ):
        # ---- resident SBUF slabs ----
        LC = sb.tile([128, C], F16, name="lc")
        RT = sb.tile([128, TL], F16, name="rt")
        ZT = sb.tile([D, TL], F16, name="zt")
        YT = sb.tile([128, NCHUNK * DY], BF16, name="yt")
        YA = sb.tile([128, NCHUNK, DY + 1], BF16, name="ya")
        WW = sb.tile([D, 2 * D], F32, name="ww")
        SSF = sb.tile([D, D], F32, name="ssf")
        SS = sb.tile([D, D], F16, name="ss")
        SSH = sb.tile([D, D], F16, name="ssh")
        OSB = sb.tile([DY + 1, TL], F32, name="osb")
        WRM = sb.tile([128, 512], BF16, name="wrm")
        EXD = sb.tile([D, 1], F32, name="exd")

        # ---- input DMAs: SP queue for the loop-critical path ----
        nc.sync.dma_start(out=WW, in_=wwt_d.ap())
        nc.sync.dma_start(out=ZT, in_=zt_d.ap())
        nc.sync.dma_start(out=LC[:D, 0:1024], in_=zc0_d.ap())
        nc.sync.dma_start(out=LC[:D, 1024:4096], in_=zc13_d.ap())
        nc.sync.dma_start(out=LC[:D, 4096:8192], in_=zc47_d.ap())
        # y rides the ACT HWDGE queue in parallel
        nc.scalar.dma_start(out=YT, in_=y_d.ap())

        # ---- ACT exp-table preload (~2.7us) during the DMA phase ----
        nc.vector.memset(EXD, 0.0)
        nc.scalar.activation(EXD, EXD, Exp)

        # ---- PE warm-up burst: flips HAM to 8/8 before the loop ----
        nc.vector.memset(WRM, 0.5)
        for _ in range(NWARM):
            wps = pp.tile([128, 512], F32, tag="pre", bufs=2, name="wps")
            nc.tensor.matmul(wps, WRM[:, 0:128], WRM, start=True, stop=True)

        # ---- DVE prelude chain (emitted in expected-readiness order;
        # the DVE queue is strict FIFO) ----
        nc.vector.memset(RT[D:128, :], -1.0)
        nc.vector.tensor_add(SSF, WW[:, 0:D], WW[:, D : 2 * D])
        nc.vector.tensor_copy(SS, SSF)                 # -> fp16
        nc.vector.tensor_scalar_mul(SSH, SSF, 0.5)     # -> fp16, 0.5*S

        # ---- RT rows 0..63 = S^T zt (matmul halves + cast; matmul free
        # dim is capped at 512 by the PSUM bank boundary) ----
        zr = pp.tile([128, TL], F32, tag="sc", bufs=2, name="zr")
        for h in range(2):
            sl = slice(h * 512, (h + 1) * 512)
            nc.tensor.matmul(zr[:D, sl], SS, ZT[:, sl], start=True, stop=True)
        nc.vector.tensor_copy(RT[:D, :], zr[:D, :])    # -> fp16

        # zs piece k: LC rows 64..127, cols 512k..512k+512 =
        #   0.5 * (zc .* (S zc))  (fp16)
        def zs_piece(k):
            sl = slice(512 * k, 512 * (k + 1))
            ps = pp.tile([128, 512], F32, tag="pre", bufs=2, name="ps")
            nc.tensor.matmul(ps[:D, :], SSH, LC[:D, sl], start=True, stop=True)
            nc.vector.tensor_mul(LC[D:128, sl], ps[:D, :], LC[:D, sl])

        zs_piece(0)
        # y_aug piece q (16 chunks): [128, 16, 33]; col 32 = 1.0
        half = NCHUNK // 4 * DY

        def ya_piece(q):
            nc.vector.tensor_copy(
                YA[:, q * 16 : (q + 1) * 16, 0:DY],
                YT[:, q * half : (q + 1) * half].rearrange(
                    "p (j d) -> p j d", d=DY
                ),
            )

        ya_piece(0)
        zs_piece(1)
        nc.vector.memset(YA[:, :, DY : DY + 1], 1.0)
        for q in range(1, 4):
            ya_piece(q)

        # ---- main loop over 64 context chunks ----
        # zs piece k (k>=2) is emitted at chunk 4(k-2)+2, needed by chunk 4k.
        o2 = pp.tile([DY + 1, TL], F32, tag="o2", name="o2")
        for j in range(NCHUNK):
            sc = pp.tile([128, TL], F32, tag="sc", bufs=2, name="sc")
            lhsT = LC[:, j * 128 : (j + 1) * 128]
            for h in range(2):
                sl = slice(h * 512, (h + 1) * 512)
                nc.tensor.matmul(sc[:, sl], lhsT, RT[:, sl], start=True, stop=True)
            th = sb.tile([128, TL], BF16, tag="th", bufs=3, name="th")
            nc.scalar.activation(th, sc, Exp)
            for h in range(2):
                sl = slice(h * 512, (h + 1) * 512)
                nc.tensor.matmul(
                    o2[:, sl],
                    YA[:, j, :],
                    th[:, sl],
                    start=(j == 0),
                    stop=(j == NCHUNK - 1),
                )
            if j % 4 == 2 and 2 + (j - 2) // 4 < NPIECE:
                zs_piece(2 + (j - 2) // 4)

        # ---- epilogue ----
        nc.vector.tensor_copy(OSB, o2)
        nc.sync.dma_start(out=out_d.ap(), in_=OSB)


_CACHED = None


def _get_nc():
    global _CACHED
    if _CACHED is None:
        nc = bacc.Bacc(
            "TRN2",
            target_bir_lowering=False,
            debug=False,
            enable_asserts=False,
        )
        with tile.TileContext(nc) as tc:
            _build_kernel_body(tc)
        nc.compile()
        _CACHED = nc
    return _CACHED


def make_in_maps(z_context, y_context, z_target, W):
    """Host-side layout prep (transpose/reshape/cast only) + sharding."""
    z_context = np.asarray(z_context, dtype=np.float32)
    y_context = np.asarray(y_context, dtype=np.float32)
    z_target = np.asarray(z_target, dtype=np.float32)
    W = np.asarray(W, dtype=np.float32)

    zcT = np.ascontiguousarray(z_context.T.astype(np.float16))  # [64, 8192]
    zc0 = np.ascontiguousarray(zcT[:, 0:1024])
    zc13 = np.ascontiguousarray(zcT[:, 1024:4096])
    zc47 = np.ascontiguousarray(zcT[:, 4096:8192])
    # chunk j partition p holds context j*128+p:
    # yck[p, j*DY+d] = y_context[j*128+p, d]
    yck = np.ascontiguousarray(
        y_context.reshape(NCHUNK, 128, DY).transpose(1, 0, 2).reshape(
            128, NCHUNK * DY
        )
    ).astype(ml_dtypes.bfloat16)
    wwt = np.ascontiguousarray(np.concatenate([W, W.T], axis=1))  # [64, 128]

    in_maps = []
    for i in range(NCORES):
        ztT = np.ascontiguousarray(
            z_target[i * TL : (i + 1) * TL].T.astype(np.float16)
        )
        in_maps.append(
            {
                "wwt": wwt,
                "ztt": ztT,
                "zc0": zc0,
                "zc13": zc13,
                "zc47": zc47,
                "yck": yck,
            }
        )
    return in_maps


def postprocess(results):
    """Gather per-core [33, TL] outputs -> full (T, DY) normalized output."""
    allT = np.concatenate([r["out"].T for r in results], axis=0)  # [T, 33]
    return (allT[:, :DY] / allT[:, DY : DY + 1]).astype(np.float32)


def run(in_maps, **kwargs):
    nc = _get_nc()
    return run_bass_kernel_spmd(nc, in_maps, core_ids=list(range(NCORES)), **kwargs)


def kernel(z_context, y_context, z_target, W):
    in_maps = make_in_maps(z_context, y_context, z_target, W)
    res = run(in_maps)
    return postprocess(res.results)
